# revision 63
# baseline (speedup 1.0000x reference)
"""BitNet transformer block on 8 Trainium2 NeuronCores (Bass/Tile).

Sharding: DP2 (batch) x TP4. Cores 0-3 -> batch 0, cores 4-7 -> batch 1.
Within each group of 4:
  - weights are ternarized host-side (exact {-1,0,1} in bf16) with the
    per-tensor absmean scales passed as a tiny f32 input,
  - each core owns 512 tokens for LN + act_quant (sequence parallel);
    quantized activations are AllGathered (bf16 carries exact ints),
  - attention is TOKEN-sharded: each core computes q (all 16 heads) for
    its own 512 tokens; k/v are computed channel-sharded (4 heads/core)
    over all tokens, dequantized, and AllGathered so every core holds
    full kT/v. Scores/PV run in S^T layout (exp without max subtraction;
    ones column in v gives the softmax denominator). The attention
    output is fully local per token, so o-quant + proj need NO
    collectives (proj uses the full ternary w_proj),
  - fc1 is column-parallel on the AllGathered LN2 activations; gelu
    output stays in SBUF as bf16; its per-token absmax is AllGathered
    (8KB) for the exact global act_quant scale; fc2 is row-parallel with
    raw integer partials ReduceScattered in bf16.
"""

import sys

for _p in ("/opt/trn_rl_repo",):
    if _p not in sys.path:
        sys.path.append(_p)

import numpy as np

_BASS = {}


def _imports():
    if _BASS:
        return _BASS
    import concourse.bass as bass
    import concourse.bass_isa as bass_isa
    import concourse.mybir as mybir
    import concourse.tile as tile
    from concourse import bacc
    from concourse.bass_utils import run_bass_kernel_spmd
    from concourse.masks import make_identity
    _BASS.update(bass=bass, bass_isa=bass_isa, mybir=mybir, tile=tile,
                 bacc=bacc, run=run_bass_kernel_spmd, mkid=make_identity)
    return _BASS

# ---- problem constants (hardcoded per spec) ----
B, N, C, H = 2, 2048, 1024, 16
HID = 4 * C
NCORES, TP = 8, 4
TOK = N // TP            # 512 tokens per core
TT_LOC = TOK // 128      # 4
TT_ALL = N // 128        # 16
DH = C // H              # 64
CS = C // TP             # 256 channel shard (k/v outputs)
HS = HID // TP           # 1024 hidden shard
P = 128
KT = C // P              # 8
EPS = 1e-5
MAGIC = 12582912.0       # 1.5 * 2**23: fp32 round-half-even trick
G4 = [[0, 1, 2, 3], [4, 5, 6, 7]]


def build_kernel(g1_trivial, g2_trivial):
    m = _imports()
    bass, bass_isa, mybir, tile, bacc = (m["bass"], m["bass_isa"], m["mybir"],
                                         m["tile"], m["bacc"])
    F32, BF16 = mybir.dt.float32, mybir.dt.bfloat16
    AX, ALU, ACTF = (mybir.AxisListType, mybir.AluOpType,
                     mybir.ActivationFunctionType)

    make_identity = m["mkid"]
    nc = bacc.Bacc("TRN2", target_bir_lowering=False, debug=False,
                   num_devices=NCORES)

    x_sh = nc.dram_tensor("x_sh", [TOK, C], F32, kind="ExternalInput")
    wqT = nc.dram_tensor("wqT", [P, KT * C], BF16, kind="ExternalInput")
    wkT = nc.dram_tensor("wkT", [P, KT * CS], BF16, kind="ExternalInput")
    wvT = nc.dram_tensor("wvT", [P, KT * CS], BF16, kind="ExternalInput")
    wpT = nc.dram_tensor("wpT", [P, KT * C], BF16, kind="ExternalInput")
    wf1T = nc.dram_tensor("wf1T", [P, KT * HS], BF16, kind="ExternalInput")
    wf2T = nc.dram_tensor("wf2T", [P, KT * C], BF16, kind="ExternalInput")
    bq = nc.dram_tensor("bq", [C], F32, kind="ExternalInput")
    bk = nc.dram_tensor("bk", [CS], F32, kind="ExternalInput")
    bv = nc.dram_tensor("bv", [CS], F32, kind="ExternalInput")
    bp = nc.dram_tensor("bp", [C], F32, kind="ExternalInput")
    bf1 = nc.dram_tensor("bf1", [HS], F32, kind="ExternalInput")
    bf2 = nc.dram_tensor("bf2", [C], F32, kind="ExternalInput")
    wmean = nc.dram_tensor("wmean", [4], F32, kind="ExternalInput")
    g1 = be1 = g2 = be2 = None
    if not g1_trivial:
        g1 = nc.dram_tensor("g1", [C], F32, kind="ExternalInput")
        be1 = nc.dram_tensor("be1", [C], F32, kind="ExternalInput")
    if not g2_trivial:
        g2 = nc.dram_tensor("g2", [C], F32, kind="ExternalInput")
        be2 = nc.dram_tensor("be2", [C], F32, kind="ExternalInput")
    onehot = nc.dram_tensor("onehot", [TP], F32, kind="ExternalInput")
    y_sh = nc.dram_tensor("y_sh", [TOK, C], F32, kind="ExternalOutput")

    with tile.TileContext(nc) as tc:
        import contextlib
        with contextlib.ExitStack() as ctx:
            dram = ctx.enter_context(tc.tile_pool(name="dram", bufs=1, space="DRAM"))
            consts = ctx.enter_context(tc.tile_pool(name="consts", bufs=1))
            wres = ctx.enter_context(tc.tile_pool(name="wres", bufs=1))
            big = ctx.enter_context(tc.tile_pool(name="big", bufs=1))
            rowp = ctx.enter_context(tc.tile_pool(name="rowp", bufs=1))
            t8 = ctx.enter_context(tc.tile_pool(name="t8", bufs=2))
            t4 = ctx.enter_context(tc.tile_pool(name="t4", bufs=2))
            t2 = ctx.enter_context(tc.tile_pool(name="t2", bufs=2))
            t1 = ctx.enter_context(tc.tile_pool(name="t1", bufs=4))
            brow = ctx.enter_context(tc.tile_pool(name="brow", bufs=1))
            sm = ctx.enter_context(tc.tile_pool(name="sm", bufs=2))
            psp = ctx.enter_context(tc.tile_pool(name="psp", bufs=2, space="PSUM"))
            psa = ctx.enter_context(tc.tile_pool(name="psa", bufs=1, space="PSUM"))

            # ---------- DRAM internal buffers ----------
            def dt(name, shape, dtype):
                return dram.tile(shape, dtype, name=name)

            HTOK = TOK // 2  # 256 tokens per AG half
            BLK = HTOK * C + 2 * HTOK  # payload + f32 scales as bf16 pairs
            ag1_in = dt("ag1_in", [2 * BLK], BF16)
            ag1_out = dt("ag1_out", [TP * 2 * BLK], BF16)
            q1_stash = dt("q1_stash", [TOK, C], BF16)
            ag2_in = [dt("ag2_in0", [BLK], BF16), dt("ag2_in1", [BLK], BF16)]
            ag2_out = [dt("ag2_out0", [TP * BLK], BF16),
                       dt("ag2_out1", [TP * BLK], BF16)]
            # merged k+v chunks per half: k = [2 kch-slices, 128, 1024 tok],
            # v = [8 tt, 128, 256 vch]
            KBLK = 2 * P * (N // 2)
            VBLK = 8 * P * CS
            KVBLK = KBLK + VBLK
            agkv_in = [dt("agkv_in0", [KVBLK], BF16),
                       dt("agkv_in1", [KVBLK], BF16)]
            agkv_out = [dt("agkv_out0", [TP * KVBLK], BF16),
                        dt("agkv_out1", [TP * KVBLK], BF16)]
            agg_in = dt("agg_in", [N], F32)
            agg_out = dt("agg_out", [TP * N], F32)
            rs2h_in = [dt("rs2h_in%d" % i, [TP * P, C], BF16)
                       for i in range(4)]
            rs2h_out = [dt("rs2h_out%d" % i, [P, C], BF16)
                        for i in range(4)]
            gq_dram = dt("gq_dram", [N, HS], BF16)

            # ---------- constants / bias rows ----------
            ones_col = consts.tile([P, 1], F32, name="ones_col")
            nc.vector.memset(ones_col[:], 1.0)
            eps_col = consts.tile([P, 1], F32, name="eps_col")
            nc.vector.memset(eps_col[:], EPS)
            ident = consts.tile([P, P], BF16, name="ident")
            make_identity(nc, ident[:])

            def bcast_row(dram_ap, n, name, pool=None, tag=None):
                if pool is None:
                    r = consts.tile([P, n], F32, name=name)
                else:
                    r = pool.tile([P, 1024], F32, name=name, tag=tag or "brow")[:, :n]
                nc.sync.dma_start(r[:], dram_ap[None, :].to_broadcast((P, n)))
                return r

            bv_row = bcast_row(bv[:], CS, "bv_row")
            bq_col = consts.tile([P, KT], F32, name="bq_col")
            nc.sync.dma_start(bq_col[:], bq[:].rearrange("(j p) -> p j", p=P))
            bk_col = consts.tile([P, 2], F32, name="bk_col")
            nc.sync.dma_start(bk_col[:], bk[:].rearrange("(j p) -> p j", p=P))
            oh_bc = consts.tile([P, TP], F32, name="oh_bc")
            nc.sync.dma_start(oh_bc[:], onehot[None, :].to_broadcast((P, TP)))
            mean_bc = consts.tile([P, 4], F32, name="mean_bc")
            nc.sync.dma_start(mean_bc[:], wmean[None, :].to_broadcast((P, 4)))

            def own_select(dst, col_g):
                tmp_os = sm.tile([P, TT_LOC], F32, tag="ownsel")
                for r in range(TP):
                    src = col_g[:, TT_LOC * r:TT_LOC * (r + 1)]
                    if r == 0:
                        nc.vector.tensor_scalar(dst, src, oh_bc[:, 0:1], None,
                                                op0=ALU.mult)
                    else:
                        nc.vector.tensor_scalar(tmp_os[:], src,
                                                oh_bc[:, r:r + 1], None,
                                                op0=ALU.mult)
                        nc.vector.tensor_tensor(dst, dst, tmp_os[:], ALU.add)

            # ---------- weights: direct ternary loads ----------
            wq_bf = wres.tile([P, KT, C], BF16, tag="wslotA")      # 2MB
            wk_bf = wres.tile([P, KT, CS], BF16, tag="wslotB1")    # 0.5MB
            wv_bf = wres.tile([P, KT, CS], BF16, tag="wslotB2")    # 0.5MB
            wp_bf = wres.tile([P, KT, C], BF16, tag="wslotC")      # 2MB
            # host pre-swizzles weights to [P, KT, cols] row-major, so these
            # are flat contiguous copies
            for (wt, dst, cols) in ((wqT, wq_bf, C), (wkT, wk_bf, CS),
                                    (wvT, wv_bf, CS), (wpT, wp_bf, C)):
                nc.scalar.dma_start(dst[:],
                                    wt[:].rearrange("p (o c) -> p o c", c=cols))

            # ---------- LN1 + act_quant (own 512 tokens) ----------
            def ln_quant(x_tile, g_row, be_row, trivial, qout_bf, m_out):
                st6 = sm.tile([P, 2, 6], F32, tag="bnst")
                nc.vector.bn_stats(st6[:, 0, :], x_tile[:, 0:C // 2])
                nc.vector.bn_stats(st6[:, 1, :], x_tile[:, C // 2:C])
                agg = sm.tile([P, 2], F32, tag="bnagg")
                nc.vector.bn_aggr(agg[:], st6[:])
                rstd = sm.tile([P, 1], F32, tag="rstd")
                nc.scalar.activation(rstd[:], agg[:, 1:2], ACTF.Sqrt, bias=eps_col[:])
                nc.vector.reciprocal(rstd[:], rstd[:])
                h = t4.tile([P, C], F32, tag="t4f32")
                nc.vector.tensor_scalar(h[:], x_tile, agg[:, 0:1], rstd[:],
                                        op0=ALU.subtract, op1=ALU.mult)
                if not trivial:
                    nc.vector.tensor_tensor(h[:], h[:], g_row[:, :C], ALU.mult)
                    nc.vector.tensor_tensor(h[:], h[:], be_row[:, :C], ALU.add)
                nc.vector.tensor_reduce(m_out, h[:], axis=AX.X, op=ALU.max,
                                        apply_absolute_value=True)
                nc.vector.tensor_scalar(m_out, m_out, EPS, None, op0=ALU.max)
                s = sm.tile([P, 1], F32, tag="qs")
                nc.vector.reciprocal(s[:], m_out)
                nc.vector.tensor_scalar(s[:], s[:], 127.0, None, op0=ALU.mult)
                nc.vector.tensor_scalar(h[:], h[:], s[:], MAGIC,
                                        op0=ALU.mult, op1=ALU.add)
                nc.scalar.activation(qout_bf, h[:], ACTF.Copy, bias=-MAGIC)

            g1_row = be1_row = None
            if not g1_trivial:
                g1_row = bcast_row(g1[:], C, "g1_row", pool=brow)
                be1_row = bcast_row(be1[:], C, "be1_row", pool=brow)
            # x kept resident for the later residual
            x_res = big.tile([P, TT_LOC, C], F32, tag="slotX")
            m1_loc = sm.tile([P, TT_LOC], F32, name="m1_loc")
            for j in range(TT_LOC):
                nc.sync.dma_start(x_res[:, j, :], x_sh[j * P:(j + 1) * P, :])
                q1t = t2.tile([P, C], BF16, tag="t2bf")
                ln_quant(x_res[:, j, :], g1_row, be1_row, g1_trivial, q1t[:],
                         m1_loc[:, j:j + 1])
                hfb = (j // 2) * BLK
                nc.sync.dma_start(
                    ag1_in[hfb:hfb + HTOK * C]
                    .rearrange("(j p c) -> p j c", p=P, c=C)[:, j % 2, :], q1t[:])
                nc.sync.dma_start(q1_stash[j * P:(j + 1) * P, :], q1t[:])
                nc.sync.dma_start(
                    ag1_in[hfb + HTOK * C:hfb + BLK].bitcast(F32)
                    .rearrange("(j p) -> p j", p=P)[:, j % 2:j % 2 + 1],
                    m1_loc[:, j:j + 1])
                if j == TT_LOC - 1:
                    nc.gpsimd.collective_compute(
                        "AllGather", ALU.bypass, replica_groups=G4,
                        ins=[ag1_in.opt()], outs=[ag1_out.opt()])

            # dequant factor rows/cols from scales (own from ag1_in, no AG dep)
            rtmp = rowp.tile([P, N], F32, tag="rowtmp")
            m1_col = sm.tile([P, TT_ALL], F32, name="m1_col")

            def scale_srcs(ag_in, ag_out):
                # yields (rank, half, own_scales, remote_scales) views
                for r in range(TP):
                    for hf in range(2):
                        base = hf * BLK + HTOK * C
                        own = ag_in[base:base + 2 * HTOK].bitcast(F32)
                        rb = r * 2 * BLK + base
                        rem = ag_out[rb:rb + 2 * HTOK].bitcast(F32)
                        yield r, hf, own, rem

            for r, hf, own_sc, rem_sc in scale_srcs(ag1_in, ag1_out):
                toff = r * TOK + hf * HTOK
                joff = r * TT_LOC + hf * 2
                # NOTE: own rank's scales read from local ag1_in to skip AG dep
                # (cannot branch on rank at trace time -> use ag1_out for all;
                #  own block of ag1_out equals ag1_in content)
                nc.sync.dma_start(rtmp[:, toff:toff + HTOK],
                                  rem_sc[None, :].to_broadcast((P, HTOK)))
                nc.sync.dma_start(m1_col[:, joff:joff + 2],
                                  rem_sc.rearrange("(j p) -> p j", p=P))
            # per-chunk factor computation so chunk-0 dequant never waits on
            # the second AG1 collective
            rinv1_bc = rtmp
            rinv1_col = sm.tile([P, TT_ALL], F32, name="rinv1_col")
            r1b4 = rinv1_bc[:].rearrange("p (r x) -> p r x", x=TOK)
            rt4 = rtmp[:].rearrange("p (r x) -> p r x", x=TOK)
            m1c4 = m1_col[:].rearrange("p (r four) -> p r four", four=4)
            r1c4 = rinv1_col[:].rearrange("p (r four) -> p r four", four=4)
            for hf in range(2):
                nc.vector.tensor_scalar(
                    r1b4[:, :, hf * HTOK:(hf + 1) * HTOK],
                    rt4[:, :, hf * HTOK:(hf + 1) * HTOK],
                    mean_bc[:, 0:1], 1.0 / 127.0, op0=ALU.mult, op1=ALU.mult)
                nc.vector.tensor_scalar(
                    r1c4[:, :, 2 * hf:2 * hf + 2],
                    m1c4[:, :, 2 * hf:2 * hf + 2],
                    mean_bc[:, 0:1], 1.0 / 127.0, op0=ALU.mult, op1=ALU.mult)

            # own-token dequant row for q (local scales, no AG dep)
            rq_own = rowp.tile([P, TOK], F32, tag="rqown")
            for hf in range(2):
                own_sc = ag1_in[hf * BLK + HTOK * C:hf * BLK + BLK].bitcast(F32)
                nc.sync.dma_start(rq_own[:, hf * HTOK:(hf + 1) * HTOK],
                                  own_sc[None, :].to_broadcast((P, HTOK)))
            nc.vector.tensor_scalar(rq_own[:], rq_own[:], mean_bc[:, 0:1],
                                    1.0 / 127.0, op0=ALU.mult, op1=ALU.mult)

            # ---------- QKV ----------
            qT_own = big.tile([P, KT, TOK], BF16, tag="slotQ")     # 1MB
            kT_all = big.tile([P, KT, N], BF16, tag="slotK")       # 4MB
            v_aug = big.tile([P, TT_ALL, H, DH + 1], BF16, tag="slotV")  # 4.2MB
            nc.vector.memset(v_aug[:, :, :, DH:DH + 1], 1.0)

            # q: full 1024 channels for OWN tokens, from local ag1_in
            q1T_own = t8.tile([P, KT, TOK], BF16, tag="t8bf", name="q1T_own")
            for hf in range(2):
                nc.sync.dma_start_transpose(
                    q1T_own[:, :, hf * HTOK:(hf + 1) * HTOK],
                    q1_stash[hf * HTOK:(hf + 1) * HTOK, :])
            for jt2 in range(KT // 2):
                pq = psp.tile([P, 2, 512], F32, tag="pb2")
                for sub in range(2):
                    jt = jt2 * 2 + sub
                    for ct in range(KT):
                        nc.tensor.matmul(pq[:, sub, :],
                                         wq_bf[:, ct, jt * P:(jt + 1) * P],
                                         q1T_own[:, ct, :], start=(ct == 0),
                                         stop=(ct == KT - 1))
                for sub in range(2):
                    jt = jt2 * 2 + sub
                    dq = t2.tile([P, 512], F32, tag="t2f32")
                    nc.vector.tensor_tensor(dq[:], pq[:, sub, :], rq_own[:],
                                            ALU.mult)
                    nc.vector.tensor_scalar(qT_own[:, jt, :], dq[:],
                                            bq_col[:, jt:jt + 1], None,
                                            op0=ALU.add)

            # k/v channel shards over ALL tokens, chunk by gathered 512-token
            # blocks; dequantized bf16 values are staged and AllGathered (own
            # block included -- SPMD cannot branch on rank at trace time).
            def rblock_src(ag_out_hf, t1c):
                return ag_out_hf[t1c * BLK:t1c * BLK + HTOK * C] \
                    .rearrange("(t c) -> t c", c=C)

            def r1block(r, hf):
                base = r * 2 * BLK + hf * BLK
                return ag1_out[base:base + HTOK * C] \
                    .rearrange("(t c) -> t c", c=C)

            # hf-outer: token block (r, hf) = tokens r*512+hf*256..+256, so kv
            # AG chunk hf only needs AG1 chunk hf.
            # agkv chunk layout: k "(o p rt t)" [2,128,4,256], v "(j p v)"
            # with j = 2*rt + sub.
            for hf in range(2):
                agk_v = agkv_in[hf][0:KBLK].rearrange(
                    "(o p rt t) -> p o rt t", p=P, rt=TP, t=HTOK)
                agv_v = agkv_in[hf][KBLK:KVBLK].rearrange("(j p v) -> p j v",
                                                          p=P, v=CS)
                for r in range(TP):
                    tsl = slice(r * 512 + hf * HTOK, r * 512 + (hf + 1) * HTOK)
                    q1T = t8.tile([P, KT, HTOK], BF16, tag="t8bf")
                    nc.sync.dma_start_transpose(q1T[:],
                                                r1block(r, hf))
                    pk = psp.tile([P, 2, 512], F32, tag="pb2")
                    for o in range(2):
                        for ct in range(KT):
                            nc.tensor.matmul(pk[:, o, 0:HTOK],
                                             wk_bf[:, ct, o * P:(o + 1) * P],
                                             q1T[:, ct, :], start=(ct == 0),
                                             stop=(ct == KT - 1))
                    for o in range(2):
                        dk = t2.tile([P, 512], F32, tag="t2f32")
                        nc.vector.tensor_tensor(dk[:, 0:HTOK], pk[:, o, 0:HTOK],
                                                rinv1_bc[:, tsl], ALU.mult)
                        kq = t1.tile([P, 512], BF16, tag="t1bf", bufs=1)
                        nc.vector.tensor_scalar(kq[:, 0:HTOK], dk[:, 0:HTOK],
                                                bk_col[:, o:o + 1], None,
                                                op0=ALU.add)
                        nc.sync.dma_start(agk_v[:, o, r, :], kq[:, 0:HTOK])
                    pv = psp.tile([P, 2, 512], F32, tag="pb2")
                    for sub in range(2):
                        tt = 4 * r + 2 * hf + sub
                        for ct in range(KT):
                            nc.tensor.matmul(pv[:, sub, 0:CS],
                                             q1T[:, ct, sub * P:(sub + 1) * P],
                                             wv_bf[:, ct, :], start=(ct == 0),
                                             stop=(ct == KT - 1))
                    for sub in range(2):
                        tt = 4 * r + 2 * hf + sub
                        vdq = t1.tile([P, CS], F32, tag="t1f32", bufs=2)
                        nc.vector.tensor_scalar(vdq[:], pv[:, sub, 0:CS],
                                                rinv1_col[:, tt:tt + 1], None,
                                                op0=ALU.mult)
                        vq = t1.tile([P, CS], BF16, tag="t1bfv", bufs=2)
                        nc.vector.tensor_tensor(vq[:], vdq[:], bv_row[:], ALU.add)
                        nc.sync.dma_start(agv_v[:, 2 * r + sub, :], vq[:])
                nc.gpsimd.collective_compute(
                    "AllGather", ALU.bypass, replica_groups=G4,
                    ins=[agkv_in[hf].opt()], outs=[agkv_out[hf].opt()])

            # gather k/v of ALL ranks into kT_all / v_aug (per AG chunk hf)
            v_aug5 = v_aug[:].rearrange("p (rt four) h d -> p rt four h d",
                                        four=4)
            for r in range(TP):      # source rank (channel shard)
                for hf in range(2):
                    base = r * KVBLK
                    ksrc = agkv_out[hf][base:base + KBLK].rearrange(
                        "(o p rt t) -> p o rt t", p=P, rt=TP, t=HTOK)
                    for o in range(2):
                        dst = kT_all[:, 2 * r + o, :].rearrange(
                            "p (rt x) -> p rt x", x=512)[:, :, hf * HTOK:(hf + 1) * HTOK]
                        nc.sync.dma_start(dst, ksrc[:, o, :, :])
                    vsrc = agkv_out[hf][base + KBLK:base + KVBLK].rearrange(
                        "(rt two p h d) -> rt two p h d", two=2, p=P, h=TP, d=DH)
                    for hh in range(TP):
                        for sub in range(2):
                            nc.sync.dma_start(
                                v_aug5[:, :, 2 * hf + sub, TP * r + hh, 0:DH],
                                vsrc[:, sub, :, hh, :].rearrange(
                                    "rt p d -> p rt d"))

            # ---------- attention (16 heads, own 512 query tokens) ----------
            o_un = big.tile([P, TT_LOC, H, DH + 1], BF16, tag="slotO")
            SCALE = DH ** -0.5
            # key tiles in kv-AG-chunk arrival order (chunk 0 tiles first)
            KT_ORDER = [4 * r + 2 * hf + sub for hf in range(2)
                        for r in range(TP) for sub in range(2)]
            # two passes (one per kv AG chunk): each pair's PSUM accumulator
            # is released at the end of its pass, so pass-0 work for all 8
            # pairs streams without waiting for the second kv chunk. Pass 0
            # writes o_un; pass 1 accumulates into it.
            for half_pass in range(2):
                for hp in range(H // 2):
                    h_e, h_o = 2 * hp, 2 * hp + 1
                    po_e = psa.tile([P, 512], F32, tag="po_e")
                    po_o = psa.tile([P, 512], F32, tag="po_o")
                    for kti8 in range(8):
                        tt2 = KT_ORDER[half_pass * 8 + kti8]
                        sreg = psp.tile([P, 2, 512], F32, tag="pb2")
                        for ii, hh in enumerate((h_e, h_o)):
                            jk = DH * hh
                            kT_ap = kT_all[(jk % P):(jk % P) + DH, jk // P,
                                           tt2 * P:(tt2 + 1) * P]
                            qT_ap = qT_own[(jk % P):(jk % P) + DH, jk // P, :]
                            nc.tensor.matmul(sreg[:, ii, :], kT_ap, qT_ap,
                                             start=True, stop=True)
                        pt = t1.tile([P, 2, 512], BF16, tag="ptbf", bufs=3)
                        nc.scalar.activation(pt[:], sreg[:], ACTF.Exp,
                                             scale=SCALE)
                        nc.tensor.matmul(po_e[0:DH + 1, :],
                                         v_aug[:, tt2, h_e, :],
                                         pt[:, 0, :], start=(kti8 == 0),
                                         stop=(kti8 == 7),
                                         skip_group_check=True)
                        nc.tensor.matmul(po_o[0:DH + 1, :],
                                         v_aug[:, tt2, h_o, :],
                                         pt[:, 1, :], start=(kti8 == 0),
                                         stop=(kti8 == 7),
                                         skip_group_check=True)
                    # evacuate this pass's partial into o_un
                    for ii, (po, hh) in enumerate(((po_e, h_e), (po_o, h_o))):
                        stg = t1.tile([DH + 1, 512], BF16, tag="postg", bufs=2)
                        nc.vector.tensor_copy(stg[:], po[0:DH + 1, :])
                        for tb in range(TT_LOC):
                            trp = psp.tile([P, 1024], BF16, tag="pbb")
                            nc.tensor.transpose(trp[:, 0:DH + 1],
                                                stg[:, tb * P:(tb + 1) * P],
                                                ident[0:DH + 1, 0:DH + 1])
                            if half_pass == 0:
                                nc.vector.tensor_copy(o_un[:, tb, hh, :],
                                                      trp[:, 0:DH + 1])
                            else:
                                nc.vector.tensor_tensor(o_un[:, tb, hh, :],
                                                        o_un[:, tb, hh, :],
                                                        trp[:, 0:DH + 1],
                                                        ALU.add)

            # ---------- o quant (fully local) + transpose back ----------
            oqT = qT_own  # reuse slotQ storage (last read: score matmuls)
            mo_col = sm.tile([P, TT_LOC], F32, name="mo_col")
            for tb in range(TT_LOC):
                linv = sm.tile([P, H], BF16, tag="linv")
                with nc.allow_low_precision(reason="1/l feeds int8 quant"):
                    nc.vector.reciprocal(linv[:], o_un[:, tb, :, DH:DH + 1]
                                         .rearrange("p h one -> p (h one)"))
                o_n = t4.tile([P, H, DH], F32, tag="t4f32", name="o_n")
                nc.vector.tensor_tensor(
                    o_n[:], o_un[:, tb, :, 0:DH],
                    linv[:, :, None].to_broadcast((P, H, DH)), ALU.mult)
                nc.vector.tensor_reduce(mo_col[:, tb:tb + 1],
                                        o_n[:].rearrange("p h d -> p (h d)"),
                                        axis=AX.X, op=ALU.max,
                                        apply_absolute_value=True)
                nc.vector.tensor_scalar(mo_col[:, tb:tb + 1],
                                        mo_col[:, tb:tb + 1], EPS, None,
                                        op0=ALU.max)
                so = sm.tile([P, 1], F32, tag="so")
                nc.vector.reciprocal(so[:], mo_col[:, tb:tb + 1])
                nc.vector.tensor_scalar(so[:], so[:], 127.0, None, op0=ALU.mult)
                qtmp = t4.tile([P, C], F32, tag="t4f32", name="qtmp")
                nc.vector.tensor_scalar(qtmp[:],
                                        o_n[:].rearrange("p h d -> p (h d)"),
                                        so[:], MAGIC, op0=ALU.mult, op1=ALU.add)
                oq_tb = t2.tile([P, C], BF16, tag="t2bf")
                nc.vector.tensor_scalar(oq_tb[:], qtmp[:], MAGIC, None,
                                        op0=ALU.subtract)
                for ct in range(KT):
                    trp = psp.tile([P, 1024], BF16, tag="pbb")
                    nc.tensor.transpose(trp[:, 0:P],
                                        oq_tb[:, ct * P:(ct + 1) * P], ident[:])
                    nc.vector.tensor_copy(oqT[:, ct, tb * P:(tb + 1) * P],
                                          trp[:, 0:P])

            # ---------- proj (local, full w_proj) + x_mid ----------
            rinvp_col = sm.tile([P, TT_LOC], F32, name="rinvp_col")
            nc.vector.tensor_scalar(rinvp_col[:], mo_col[:], mean_bc[:, 1:2],
                                    1.0 / 127.0, op0=ALU.mult, op1=ALU.mult)
            bp_row = bcast_row(bp[:], C, "bp_row", pool=brow)
            x_mid = x_res  # accumulate in place
            for tb in range(TT_LOC):
                nc.vector.tensor_tensor(x_mid[:, tb, :], x_mid[:, tb, :],
                                        bp_row[:, :C], ALU.add)
                pp = psp.tile([P, 2, 512], F32, tag="pb2")
                for half in range(2):
                    for ct in range(KT):
                        nc.tensor.matmul(pp[:, half, :],
                                         oqT[:, ct, tb * P:(tb + 1) * P],
                                         wp_bf[:, ct, half * 512:(half + 1) * 512],
                                         start=(ct == 0), stop=(ct == KT - 1))
                for half in range(2):
                    pdq = t2.tile([P, 512], F32, tag="t2f32")
                    nc.vector.tensor_scalar(pdq[:], pp[:, half, :],
                                            rinvp_col[:, tb:tb + 1], None,
                                            op0=ALU.mult)
                    nc.vector.tensor_tensor(x_mid[:, tb, half * 512:(half + 1) * 512],
                                            x_mid[:, tb, half * 512:(half + 1) * 512],
                                            pdq[:], ALU.add)

            # fc weights (loads overlap attention; alias early slots)
            wf1_bf = wres.tile([P, KT, HS], BF16, tag="wslotA")
            wf2_bf = wres.tile([P, KT, C], BF16, tag="wslotC")
            nc.scalar.dma_start(wf1_bf[:],
                                wf1T[:].rearrange("p (o c) -> p o c", c=HS))
            nc.scalar.dma_start(wf2_bf[:],
                                wf2T[:].rearrange("p (o c) -> p o c", c=C))

            # ---------- LN2 + quant + AG2 ----------
            g2_row = be2_row = None
            if not g2_trivial:
                g2_row = bcast_row(g2[:], C, "g2_row", pool=brow)
                be2_row = bcast_row(be2[:], C, "be2_row", pool=brow)
            m2_loc = sm.tile([P, TT_LOC], F32, name="m2_loc")
            for j in range(TT_LOC):
                q2t = t2.tile([P, C], BF16, tag="t2bf")
                ln_quant(x_mid[:, j, :], g2_row, be2_row, g2_trivial, q2t[:],
                         m2_loc[:, j:j + 1])
                nc.sync.dma_start(
                    ag2_in[j // 2][0:HTOK * C]
                    .rearrange("(j p c) -> p j c", p=P, c=C)[:, j % 2, :], q2t[:])
                nc.sync.dma_start(
                    ag2_in[j // 2][HTOK * C:BLK].bitcast(F32)
                    .rearrange("(j p) -> p j", p=P)[:, j % 2:j % 2 + 1],
                    m2_loc[:, j:j + 1])
                if j % 2 == 1:
                    nc.gpsimd.collective_compute(
                        "AllGather", ALU.bypass, replica_groups=G4,
                        ins=[ag2_in[j // 2].opt()],
                        outs=[ag2_out[j // 2].opt()])

            rinv2_col = sm.tile([P, TT_ALL], F32, name="rinv2_col")
            r2c4 = rinv2_col[:].rearrange("p (r four) -> p r four", four=4)
            for hf in range(2):
                for r in range(TP):
                    sc_r = ag2_out[hf][r * BLK + HTOK * C:(r + 1) * BLK].bitcast(F32)
                    joff = r * TT_LOC + hf * 2
                    nc.sync.dma_start(rinv2_col[:, joff:joff + 2],
                                      sc_r.rearrange("(j p) -> p j", p=P))
                nc.vector.tensor_scalar(
                    r2c4[:, :, 2 * hf:2 * hf + 2],
                    r2c4[:, :, 2 * hf:2 * hf + 2],
                    mean_bc[:, 2:3], 1.0 / 127.0, op0=ALU.mult, op1=ALU.mult)

            # ---------- fc1 + gelu (bf16, SBUF-resident) ----------
            bf1_row = bcast_row(bf1[:], HS, "bf1_row", pool=brow)
            bf1_bf = consts.tile([P, HS], BF16, name="bf1_bf")
            nc.vector.tensor_copy(bf1_bf[:], bf1_row[:, :HS])
            gres = kT_all.bitcast(BF16).rearrange("p o n -> p (o n)") \
                .rearrange("p (t h) -> p t h", h=HS)  # alias slotK as [P,16,HS]
            mg_col = sm.tile([P, TT_ALL], F32, name="mg_col")
            # process per AG2 half (hf), per rank block (256 tokens = 2 tiles)
            for hf in range(2):
                for r in range(TP):
                    q2T = t8.tile([P, KT, HTOK], BF16, tag="t8bf")
                    nc.sync.dma_start_transpose(q2T[:],
                                                rblock_src(ag2_out[hf], r))
                    for sub in range(2):
                        tt = r * TT_LOC + hf * 2 + sub
                        gt = gres[:, tt, :]
                        gparts = sm.tile([P, 2], F32, tag="gparts")
                        ph = psp.tile([P, 2, 512], F32, tag="pb2")
                        for half in range(2):
                            for ct in range(KT):
                                nc.tensor.matmul(
                                    ph[:, half, :],
                                    q2T[:, ct, sub * P:(sub + 1) * P],
                                    wf1_bf[:, ct, half * 512:(half + 1) * 512],
                                    start=(ct == 0), stop=(ct == KT - 1))
                        for half in range(2):
                            hsl = slice(half * 512, (half + 1) * 512)
                            gdq = t2.tile([P, 512], BF16, tag="t2bfb")
                            nc.vector.tensor_scalar(gdq[:], ph[:, half, :],
                                                    rinv2_col[:, tt:tt + 1],
                                                    None, op0=ALU.mult)
                            nc.vector.tensor_tensor(gdq[:], gdq[:],
                                                    bf1_bf[:, hsl], ALU.add)
                            nc.scalar.activation(gt[:, hsl], gdq[:], ACTF.Gelu)
                            nc.vector.tensor_reduce(gparts[:, half:half + 1],
                                                    gt[:, hsl], axis=AX.X,
                                                    op=ALU.max,
                                                    apply_absolute_value=True)
                        nc.vector.tensor_reduce(mg_col[:, tt:tt + 1], gparts[:],
                                                axis=AX.X, op=ALU.max)
            nc.vector.tensor_scalar(mg_col[:], mg_col[:], EPS, None, op0=ALU.max)
            nc.sync.dma_start(agg_in[:].rearrange("(j p) -> p j", p=P), mg_col[:])
            nc.gpsimd.collective_compute(
                "AllGather", ALU.bypass, replica_groups=G4,
                ins=[agg_in.opt()], outs=[agg_out.opt()])
            mg_all = sm.tile([P, TT_ALL, TP], F32, name="mg_all")
            for r in range(TP):
                nc.sync.dma_start(
                    mg_all[:, :, r],
                    agg_out[r * N:(r + 1) * N].rearrange("(j p) -> p j", p=P))
            mg_colg = sm.tile([P, TT_ALL], F32, name="mg_colg")
            nc.vector.tensor_reduce(mg_colg[:], mg_all[:], axis=AX.X, op=ALU.max)

            # requant with global scale (in SBUF), spill bf16 ints for fc2
            sg_col = sm.tile([P, TT_ALL], F32, name="sg_col")
            nc.vector.reciprocal(sg_col[:], mg_colg[:])
            nc.vector.tensor_scalar(sg_col[:], sg_col[:], 127.0, None,
                                    op0=ALU.mult)
            for tt in range(TT_ALL):
                qf = t4.tile([P, HS], F32, tag="t4f32")
                nc.vector.tensor_scalar(qf[:], gres[:, tt, :],
                                        sg_col[:, tt:tt + 1], MAGIC,
                                        op0=ALU.mult, op1=ALU.add)
                nc.vector.tensor_scalar(gres[:, tt, :], qf[:], MAGIC, None,
                                        op0=ALU.subtract)
                nc.sync.dma_start(gq_dram[tt * P:(tt + 1) * P, :], gres[:, tt, :])

            # ---------- fc2 (raw int partials, chunked RS) ----------
            for h2 in range(2):
                for r in range(TP):
                    t0 = (TP * r + 2 * h2) * P
                    gT = t8.tile([P, HS // P, HTOK], BF16, tag="t8bf")
                    nc.sync.dma_start_transpose(gT[:], gq_dram[t0:t0 + HTOK, :])
                    for w in range(2):
                        pf = psp.tile([P, 2, 512], F32, tag="pb2")
                        for half in range(2):
                            for ct in range(HS // P):
                                nc.tensor.matmul(
                                    pf[:, half, :], gT[:, ct, w * P:(w + 1) * P],
                                    wf2_bf[:, ct, half * 512:(half + 1) * 512],
                                    start=(ct == 0), stop=(ct == HS // P - 1))
                        fcp = t1.tile([P, 2, 512], BF16, tag="t1bf2", bufs=2)
                        nc.vector.tensor_copy(fcp[:], pf[:])
                        nc.sync.dma_start(
                            rs2h_in[2 * h2 + w][r * P:(r + 1) * P, :],
                            fcp[:].rearrange("p two c -> p (two c)"))
                for w in range(2):
                    nc.gpsimd.collective_compute(
                        "ReduceScatter", ALU.add, replica_groups=G4,
                        ins=[rs2h_in[2 * h2 + w].opt()],
                        outs=[rs2h_out[2 * h2 + w].opt()])

            # ---------- final: y = x_mid + deq(rs2) + bf2 ----------
            bf2_row = bcast_row(bf2[:], C, "bf2_row", pool=brow)
            rinvg_own = sm.tile([P, TT_LOC], F32, name="rinvg_own")
            own_select(rinvg_own[:], mg_colg[:])
            nc.vector.tensor_scalar(rinvg_own[:], rinvg_own[:],
                                    mean_bc[:, 3:4], 1.0 / 127.0,
                                    op0=ALU.mult, op1=ALU.mult)
            for j in range(TT_LOC):
                rst = t2.tile([P, C], BF16, tag="t2bf")
                nc.sync.dma_start(rst[:], rs2h_out[j][:, :])
                yt = t4.tile([P, C], F32, tag="t4f32")
                nc.vector.tensor_scalar(yt[:], rst[:], rinvg_own[:, j:j + 1],
                                        None, op0=ALU.mult)
                nc.vector.tensor_tensor(yt[:], yt[:], bf2_row[:, :C], ALU.add)
                nc.vector.tensor_tensor(yt[:], yt[:], x_mid[:, j, :], ALU.add)
                nc.sync.dma_start(y_sh[j * P:(j + 1) * P, :], yt[:])

    nc.compile()
    return nc


_CACHE = {}


def _ternary(w, scale):
    import ml_dtypes
    return np.clip(np.round(w * scale), -1, 1).astype(ml_dtypes.bfloat16)


def kernel(**inputs):
    m = _imports()
    x = np.ascontiguousarray(np.asarray(inputs["x"]), dtype=np.float32)
    assert int(inputs["num_heads"]) == H
    w_qkv = np.asarray(inputs["w_qkv"], np.float32)
    b_qkv = np.asarray(inputs["b_qkv"], np.float32)
    w_proj = np.asarray(inputs["w_proj"], np.float32)
    b_proj = np.asarray(inputs["b_proj"], np.float32)
    w_fc1 = np.asarray(inputs["w_fc1"], np.float32)
    b_fc1 = np.asarray(inputs["b_fc1"], np.float32)
    w_fc2 = np.asarray(inputs["w_fc2"], np.float32)
    b_fc2 = np.asarray(inputs["b_fc2"], np.float32)
    g1 = np.asarray(inputs["g1"], np.float32)
    be1 = np.asarray(inputs["be1"], np.float32)
    g2 = np.asarray(inputs["g2"], np.float32)
    be2 = np.asarray(inputs["be2"], np.float32)

    g1_trivial = bool(np.all(g1 == 1.0) and np.all(be1 == 0.0))
    g2_trivial = bool(np.all(g2 == 1.0) and np.all(be2 == 0.0))

    key = (g1_trivial, g2_trivial)
    if key not in _CACHE:
        _CACHE[key] = build_kernel(g1_trivial, g2_trivial)
    nc = _CACHE[key]

    # host-side per-tensor absmean scales + ternarization
    means = np.array([max(np.abs(w_qkv).mean(), EPS),
                      max(np.abs(w_proj).mean(), EPS),
                      max(np.abs(w_fc1).mean(), EPS),
                      max(np.abs(w_fc2).mean(), EPS)], np.float32)
    wq_t = _ternary(w_qkv[0:C, :].T, 1.0 / means[0])          # [C, C]
    wk_t = _ternary(w_qkv[C:2 * C, :].T, 1.0 / means[0])      # [C, C]
    wv_t = _ternary(w_qkv[2 * C:, :].T, 1.0 / means[0])       # [C, C]
    wp_t = _ternary(w_proj.T, 1.0 / means[1])                 # [C, C]
    wf1_t = _ternary(w_fc1.T, 1.0 / means[2])                 # [C, HID]
    wf2_t = _ternary(w_fc2.T, 1.0 / means[3])                 # [HID, C]

    def _swz(wT):
        # [(KT*P), cols] -> [P, KT*cols] row-major (device reads it flat)
        cols = wT.shape[1]
        return np.ascontiguousarray(
            wT.reshape(KT, P, cols).transpose(1, 0, 2).reshape(P, KT * cols))

    in_maps = []
    for c in range(NCORES):
        g, r = divmod(c, TP)
        tok = slice(TOK * r, TOK * (r + 1))
        hsl = slice(CS * r, CS * (r + 1))
        im = {
            "x_sh": np.ascontiguousarray(x[g, tok]),
            "wqT": _swz(wq_t),
            "wkT": _swz(wk_t[:, hsl]),
            "wvT": _swz(wv_t[:, hsl]),
            "wpT": _swz(wp_t),
            "wf1T": _swz(wf1_t[:, HS * r:HS * (r + 1)]),
            "wf2T": _swz(wf2_t[HS * r:HS * (r + 1), :]),
            "bq": np.ascontiguousarray(b_qkv[0:C]),
            "bk": np.ascontiguousarray(b_qkv[C:2 * C][hsl]),
            "bv": np.ascontiguousarray(b_qkv[2 * C:][hsl]),
            "bp": b_proj,
            "bf1": np.ascontiguousarray(b_fc1[HS * r:HS * (r + 1)]),
            "bf2": b_fc2,
            "wmean": means,
            "onehot": np.eye(TP, dtype=np.float32)[r],
        }
        if not g1_trivial:
            im["g1"], im["be1"] = g1, be1
        if not g2_trivial:
            im["g2"], im["be2"] = g2, be2
        in_maps.append(im)

    global _last_in_maps
    _last_in_maps = in_maps
    res = m["run"](nc, in_maps, core_ids=list(range(NCORES)))
    out = np.empty((B, N, C), np.float32)
    for c in range(NCORES):
        g, r = divmod(c, TP)
        out[g, TOK * r:TOK * (r + 1)] = res.results[c]["y_sh"]
    return out


# revision 64
# speedup vs baseline: 1.0391x; 1.0391x over previous
"""BitNet transformer block on 8 Trainium2 NeuronCores (Bass/Tile).

Sharding: DP2 (batch) x TP4. Cores 0-3 -> batch 0, cores 4-7 -> batch 1.
Within each group of 4:
  - weights are ternarized host-side (exact {-1,0,1} in bf16) with the
    per-tensor absmean scales passed as a tiny f32 input,
  - each core owns 512 tokens for LN + act_quant (sequence parallel);
    quantized activations are AllGathered (bf16 carries exact ints),
  - attention is TOKEN-sharded: each core computes q (all 16 heads) for
    its own 512 tokens; k/v are computed channel-sharded (4 heads/core)
    over all tokens, dequantized, and AllGathered so every core holds
    full kT/v. Scores/PV run in S^T layout (exp without max subtraction;
    ones column in v gives the softmax denominator). The attention
    output is fully local per token, so o-quant + proj need NO
    collectives (proj uses the full ternary w_proj),
  - fc1 is column-parallel on the AllGathered LN2 activations; gelu
    output stays in SBUF as bf16; its per-token absmax is AllGathered
    (8KB) for the exact global act_quant scale; fc2 is row-parallel with
    raw integer partials ReduceScattered in bf16.
"""

import sys

for _p in ("/opt/trn_rl_repo",):
    if _p not in sys.path:
        sys.path.append(_p)

import numpy as np

_BASS = {}


def _imports():
    if _BASS:
        return _BASS
    import concourse.bass as bass
    import concourse.bass_isa as bass_isa
    import concourse.mybir as mybir
    import concourse.tile as tile
    from concourse import bacc
    from concourse.bass_utils import run_bass_kernel_spmd
    from concourse.masks import make_identity
    _BASS.update(bass=bass, bass_isa=bass_isa, mybir=mybir, tile=tile,
                 bacc=bacc, run=run_bass_kernel_spmd, mkid=make_identity)
    return _BASS

# ---- problem constants (hardcoded per spec) ----
B, N, C, H = 2, 2048, 1024, 16
HID = 4 * C
NCORES, TP = 8, 4
TOK = N // TP            # 512 tokens per core
TT_LOC = TOK // 128      # 4
TT_ALL = N // 128        # 16
DH = C // H              # 64
CS = C // TP             # 256 channel shard (k/v outputs)
HS = HID // TP           # 1024 hidden shard
P = 128
KT = C // P              # 8
EPS = 1e-5
MAGIC = 12582912.0       # 1.5 * 2**23: fp32 round-half-even trick
G4 = [[0, 1, 2, 3], [4, 5, 6, 7]]


def build_kernel(g1_trivial, g2_trivial):
    m = _imports()
    bass, bass_isa, mybir, tile, bacc = (m["bass"], m["bass_isa"], m["mybir"],
                                         m["tile"], m["bacc"])
    F32, BF16 = mybir.dt.float32, mybir.dt.bfloat16
    AX, ALU, ACTF = (mybir.AxisListType, mybir.AluOpType,
                     mybir.ActivationFunctionType)

    make_identity = m["mkid"]
    nc = bacc.Bacc("TRN2", target_bir_lowering=False, debug=False,
                   num_devices=NCORES)

    x_sh = nc.dram_tensor("x_sh", [TOK, C], F32, kind="ExternalInput")
    wqT = nc.dram_tensor("wqT", [P, KT * C], BF16, kind="ExternalInput")
    wkT = nc.dram_tensor("wkT", [P, KT * CS], BF16, kind="ExternalInput")
    wvT = nc.dram_tensor("wvT", [P, KT * CS], BF16, kind="ExternalInput")
    wpT = nc.dram_tensor("wpT", [P, KT * C], BF16, kind="ExternalInput")
    wf1T = nc.dram_tensor("wf1T", [P, KT * HS], BF16, kind="ExternalInput")
    wf2T = nc.dram_tensor("wf2T", [P, KT * C], BF16, kind="ExternalInput")
    bq = nc.dram_tensor("bq", [C], F32, kind="ExternalInput")
    bk = nc.dram_tensor("bk", [CS], F32, kind="ExternalInput")
    bv = nc.dram_tensor("bv", [CS], F32, kind="ExternalInput")
    bp = nc.dram_tensor("bp", [C], F32, kind="ExternalInput")
    bf1 = nc.dram_tensor("bf1", [HS], F32, kind="ExternalInput")
    bf2 = nc.dram_tensor("bf2", [C], F32, kind="ExternalInput")
    wmean = nc.dram_tensor("wmean", [4], F32, kind="ExternalInput")
    g1 = be1 = g2 = be2 = None
    if not g1_trivial:
        g1 = nc.dram_tensor("g1", [C], F32, kind="ExternalInput")
        be1 = nc.dram_tensor("be1", [C], F32, kind="ExternalInput")
    if not g2_trivial:
        g2 = nc.dram_tensor("g2", [C], F32, kind="ExternalInput")
        be2 = nc.dram_tensor("be2", [C], F32, kind="ExternalInput")
    onehot = nc.dram_tensor("onehot", [TP], F32, kind="ExternalInput")
    y_sh = nc.dram_tensor("y_sh", [TOK, C], F32, kind="ExternalOutput")

    with tile.TileContext(nc) as tc:
        import contextlib
        with contextlib.ExitStack() as ctx:
            dram = ctx.enter_context(tc.tile_pool(name="dram", bufs=1, space="DRAM"))
            consts = ctx.enter_context(tc.tile_pool(name="consts", bufs=1))
            wres = ctx.enter_context(tc.tile_pool(name="wres", bufs=1))
            big = ctx.enter_context(tc.tile_pool(name="big", bufs=1))
            rowp = ctx.enter_context(tc.tile_pool(name="rowp", bufs=1))
            t8 = ctx.enter_context(tc.tile_pool(name="t8", bufs=2))
            t4 = ctx.enter_context(tc.tile_pool(name="t4", bufs=2))
            t2 = ctx.enter_context(tc.tile_pool(name="t2", bufs=2))
            t1 = ctx.enter_context(tc.tile_pool(name="t1", bufs=4))
            brow = ctx.enter_context(tc.tile_pool(name="brow", bufs=1))
            sm = ctx.enter_context(tc.tile_pool(name="sm", bufs=2))
            psp = ctx.enter_context(tc.tile_pool(name="psp", bufs=2, space="PSUM"))
            psa = ctx.enter_context(tc.tile_pool(name="psa", bufs=1, space="PSUM"))

            # ---------- DRAM internal buffers ----------
            def dt(name, shape, dtype):
                return dram.tile(shape, dtype, name=name)

            HTOK = TOK // 2  # 256 tokens per AG half
            BLK = HTOK * C + 2 * HTOK  # payload + f32 scales as bf16 pairs
            ag1_in = [dt("ag1_in0", [BLK], BF16), dt("ag1_in1", [BLK], BF16)]
            ag1_out = [dt("ag1_out0", [TP * BLK], BF16),
                       dt("ag1_out1", [TP * BLK], BF16)]
            q1_stash = dt("q1_stash", [TOK, C], BF16)
            ag2_in = [dt("ag2_in0", [BLK], BF16), dt("ag2_in1", [BLK], BF16)]
            ag2_out = [dt("ag2_out0", [TP * BLK], BF16),
                       dt("ag2_out1", [TP * BLK], BF16)]
            # merged k+v chunks per half: k = [2 kch-slices, 128, 1024 tok],
            # v = [8 tt, 128, 256 vch]
            KBLK = 2 * P * (N // 2)
            VBLK = 8 * P * CS
            KVBLK = KBLK + VBLK
            agkv_in = [dt("agkv_in0", [KVBLK], BF16),
                       dt("agkv_in1", [KVBLK], BF16)]
            agkv_out = [dt("agkv_out0", [TP * KVBLK], BF16),
                        dt("agkv_out1", [TP * KVBLK], BF16)]
            agg_in = dt("agg_in", [N], F32)
            agg_out = dt("agg_out", [TP * N], F32)
            rs2h_in = [dt("rs2h_in%d" % i, [TP * P, C], BF16)
                       for i in range(4)]
            rs2h_out = [dt("rs2h_out%d" % i, [P, C], BF16)
                        for i in range(4)]
            gq_dram = dt("gq_dram", [N, HS], BF16)

            # ---------- constants / bias rows ----------
            ones_col = consts.tile([P, 1], F32, name="ones_col")
            nc.vector.memset(ones_col[:], 1.0)
            eps_col = consts.tile([P, 1], F32, name="eps_col")
            nc.vector.memset(eps_col[:], EPS)
            ident = consts.tile([P, P], BF16, name="ident")
            make_identity(nc, ident[:])

            def bcast_row(dram_ap, n, name, pool=None, tag=None):
                if pool is None:
                    r = consts.tile([P, n], F32, name=name)
                else:
                    r = pool.tile([P, 1024], F32, name=name, tag=tag or "brow")[:, :n]
                nc.sync.dma_start(r[:], dram_ap[None, :].to_broadcast((P, n)))
                return r

            bv_row = bcast_row(bv[:], CS, "bv_row")
            bq_col = consts.tile([P, KT], F32, name="bq_col")
            nc.sync.dma_start(bq_col[:], bq[:].rearrange("(j p) -> p j", p=P))
            bk_col = consts.tile([P, 2], F32, name="bk_col")
            nc.sync.dma_start(bk_col[:], bk[:].rearrange("(j p) -> p j", p=P))
            oh_bc = consts.tile([P, TP], F32, name="oh_bc")
            nc.sync.dma_start(oh_bc[:], onehot[None, :].to_broadcast((P, TP)))
            mean_bc = consts.tile([P, 4], F32, name="mean_bc")
            nc.sync.dma_start(mean_bc[:], wmean[None, :].to_broadcast((P, 4)))

            def own_select(dst, col_g):
                tmp_os = sm.tile([P, TT_LOC], F32, tag="ownsel")
                for r in range(TP):
                    src = col_g[:, TT_LOC * r:TT_LOC * (r + 1)]
                    if r == 0:
                        nc.vector.tensor_scalar(dst, src, oh_bc[:, 0:1], None,
                                                op0=ALU.mult)
                    else:
                        nc.vector.tensor_scalar(tmp_os[:], src,
                                                oh_bc[:, r:r + 1], None,
                                                op0=ALU.mult)
                        nc.vector.tensor_tensor(dst, dst, tmp_os[:], ALU.add)

            # ---------- weights: direct ternary loads ----------
            wq_bf = wres.tile([P, KT, C], BF16, tag="wslotA")      # 2MB
            wk_bf = wres.tile([P, KT, CS], BF16, tag="wslotB1")    # 0.5MB
            wv_bf = wres.tile([P, KT, CS], BF16, tag="wslotB2")    # 0.5MB
            wp_bf = wres.tile([P, KT, C], BF16, tag="wslotC")      # 2MB
            # host pre-swizzles weights to [P, KT, cols] row-major, so these
            # are flat contiguous copies
            for (wt, dst, cols) in ((wqT, wq_bf, C), (wkT, wk_bf, CS),
                                    (wvT, wv_bf, CS), (wpT, wp_bf, C)):
                nc.scalar.dma_start(dst[:],
                                    wt[:].rearrange("p (o c) -> p o c", c=cols))

            # ---------- LN1 + act_quant (own 512 tokens) ----------
            def ln_quant(x_tile, g_row, be_row, trivial, qout_bf, m_out):
                st6 = sm.tile([P, 2, 6], F32, tag="bnst")
                nc.vector.bn_stats(st6[:, 0, :], x_tile[:, 0:C // 2])
                nc.vector.bn_stats(st6[:, 1, :], x_tile[:, C // 2:C])
                agg = sm.tile([P, 2], F32, tag="bnagg")
                nc.vector.bn_aggr(agg[:], st6[:])
                rstd = sm.tile([P, 1], F32, tag="rstd")
                nc.scalar.activation(rstd[:], agg[:, 1:2], ACTF.Sqrt, bias=eps_col[:])
                nc.vector.reciprocal(rstd[:], rstd[:])
                h = t4.tile([P, C], F32, tag="t4f32")
                nc.vector.tensor_scalar(h[:], x_tile, agg[:, 0:1], rstd[:],
                                        op0=ALU.subtract, op1=ALU.mult)
                if not trivial:
                    nc.vector.tensor_tensor(h[:], h[:], g_row[:, :C], ALU.mult)
                    nc.vector.tensor_tensor(h[:], h[:], be_row[:, :C], ALU.add)
                nc.vector.tensor_reduce(m_out, h[:], axis=AX.X, op=ALU.max,
                                        apply_absolute_value=True)
                nc.vector.tensor_scalar(m_out, m_out, EPS, None, op0=ALU.max)
                s = sm.tile([P, 1], F32, tag="qs")
                nc.vector.reciprocal(s[:], m_out)
                nc.vector.tensor_scalar(s[:], s[:], 127.0, None, op0=ALU.mult)
                nc.vector.tensor_scalar(h[:], h[:], s[:], MAGIC,
                                        op0=ALU.mult, op1=ALU.add)
                nc.scalar.activation(qout_bf, h[:], ACTF.Copy, bias=-MAGIC)

            g1_row = be1_row = None
            if not g1_trivial:
                g1_row = bcast_row(g1[:], C, "g1_row", pool=brow)
                be1_row = bcast_row(be1[:], C, "be1_row", pool=brow)
            # x kept resident for the later residual
            x_res = big.tile([P, TT_LOC, C], F32, tag="slotX")
            m1_loc = sm.tile([P, TT_LOC], F32, name="m1_loc")
            for j in range(TT_LOC):
                nc.sync.dma_start(x_res[:, j, :], x_sh[j * P:(j + 1) * P, :])
                q1t = t2.tile([P, C], BF16, tag="t2bf")
                ln_quant(x_res[:, j, :], g1_row, be1_row, g1_trivial, q1t[:],
                         m1_loc[:, j:j + 1])
                nc.sync.dma_start(
                    ag1_in[j // 2][0:HTOK * C]
                    .rearrange("(j p c) -> p j c", p=P, c=C)[:, j % 2, :], q1t[:])
                nc.sync.dma_start(q1_stash[j * P:(j + 1) * P, :], q1t[:])
                nc.sync.dma_start(
                    ag1_in[j // 2][HTOK * C:BLK].bitcast(F32)
                    .rearrange("(j p) -> p j", p=P)[:, j % 2:j % 2 + 1],
                    m1_loc[:, j:j + 1])
                if j % 2 == 1:
                    nc.gpsimd.collective_compute(
                        "AllGather", ALU.bypass, replica_groups=G4,
                        ins=[ag1_in[j // 2].opt()],
                        outs=[ag1_out[j // 2].opt()])

            # dequant factor rows/cols from scales (own from ag1_in, no AG dep)
            rtmp = rowp.tile([P, N], F32, tag="rowtmp")
            m1_col = sm.tile([P, TT_ALL], F32, name="m1_col")

            def scale_srcs(ag_in, ag_out):
                # yields (token_offset, tile_offset, dram_f32_scales[HTOK])
                for r in range(TP):
                    for hf in range(2):
                        own = ag_in[hf][HTOK * C:BLK].bitcast(F32)
                        rem = ag_out[hf][r * BLK + HTOK * C:(r + 1) * BLK].bitcast(F32)
                        yield r, hf, own, rem

            for r, hf, own_sc, rem_sc in scale_srcs(ag1_in, ag1_out):
                toff = r * TOK + hf * HTOK
                joff = r * TT_LOC + hf * 2
                # NOTE: own rank's scales read from local ag1_in to skip AG dep
                # (cannot branch on rank at trace time -> use ag1_out for all;
                #  own block of ag1_out equals ag1_in content)
                nc.sync.dma_start(rtmp[:, toff:toff + HTOK],
                                  rem_sc[None, :].to_broadcast((P, HTOK)))
                nc.sync.dma_start(m1_col[:, joff:joff + 2],
                                  rem_sc.rearrange("(j p) -> p j", p=P))
            # per-chunk factor computation so chunk-0 dequant never waits on
            # the second AG1 collective
            rinv1_bc = rtmp
            rinv1_col = sm.tile([P, TT_ALL], F32, name="rinv1_col")
            r1b4 = rinv1_bc[:].rearrange("p (r x) -> p r x", x=TOK)
            rt4 = rtmp[:].rearrange("p (r x) -> p r x", x=TOK)
            m1c4 = m1_col[:].rearrange("p (r four) -> p r four", four=4)
            r1c4 = rinv1_col[:].rearrange("p (r four) -> p r four", four=4)
            for hf in range(2):
                nc.vector.tensor_scalar(
                    r1b4[:, :, hf * HTOK:(hf + 1) * HTOK],
                    rt4[:, :, hf * HTOK:(hf + 1) * HTOK],
                    mean_bc[:, 0:1], 1.0 / 127.0, op0=ALU.mult, op1=ALU.mult)
                nc.vector.tensor_scalar(
                    r1c4[:, :, 2 * hf:2 * hf + 2],
                    m1c4[:, :, 2 * hf:2 * hf + 2],
                    mean_bc[:, 0:1], 1.0 / 127.0, op0=ALU.mult, op1=ALU.mult)

            # own-token dequant row for q (local scales, no AG dep)
            rq_own = rowp.tile([P, TOK], F32, tag="rqown")
            for hf in range(2):
                own_sc = ag1_in[hf][HTOK * C:BLK].bitcast(F32)
                nc.sync.dma_start(rq_own[:, hf * HTOK:(hf + 1) * HTOK],
                                  own_sc[None, :].to_broadcast((P, HTOK)))
            nc.vector.tensor_scalar(rq_own[:], rq_own[:], mean_bc[:, 0:1],
                                    1.0 / 127.0, op0=ALU.mult, op1=ALU.mult)

            # ---------- QKV ----------
            qT_own = big.tile([P, KT, TOK], BF16, tag="slotQ")     # 1MB
            kT_all = big.tile([P, KT, N], BF16, tag="slotK")       # 4MB
            v_aug = big.tile([P, TT_ALL, H, DH + 1], BF16, tag="slotV")  # 4.2MB
            nc.vector.memset(v_aug[:, :, :, DH:DH + 1], 1.0)

            # q: full 1024 channels for OWN tokens, from local ag1_in
            q1T_own = t8.tile([P, KT, TOK], BF16, tag="t8bf", name="q1T_own")
            for hf in range(2):
                nc.sync.dma_start_transpose(
                    q1T_own[:, :, hf * HTOK:(hf + 1) * HTOK],
                    q1_stash[hf * HTOK:(hf + 1) * HTOK, :])
            for jt2 in range(KT // 2):
                pq = psp.tile([P, 2, 512], F32, tag="pb2")
                for sub in range(2):
                    jt = jt2 * 2 + sub
                    for ct in range(KT):
                        nc.tensor.matmul(pq[:, sub, :],
                                         wq_bf[:, ct, jt * P:(jt + 1) * P],
                                         q1T_own[:, ct, :], start=(ct == 0),
                                         stop=(ct == KT - 1))
                for sub in range(2):
                    jt = jt2 * 2 + sub
                    dq = t2.tile([P, 512], F32, tag="t2f32")
                    nc.vector.tensor_tensor(dq[:], pq[:, sub, :], rq_own[:],
                                            ALU.mult)
                    nc.vector.tensor_scalar(qT_own[:, jt, :], dq[:],
                                            bq_col[:, jt:jt + 1], None,
                                            op0=ALU.add)

            # k/v channel shards over ALL tokens, chunk by gathered 512-token
            # blocks; dequantized bf16 values are staged and AllGathered (own
            # block included -- SPMD cannot branch on rank at trace time).
            def rblock_src(ag_out_hf, t1c):
                return ag_out_hf[t1c * BLK:t1c * BLK + HTOK * C] \
                    .rearrange("(t c) -> t c", c=C)

            # hf-outer: token block (r, hf) = tokens r*512+hf*256..+256, so kv
            # AG chunk hf only needs AG1 chunk hf.
            # agkv chunk layout: k "(o p rt t)" [2,128,4,256], v "(j p v)"
            # with j = 2*rt + sub.
            for hf in range(2):
                agk_v = agkv_in[hf][0:KBLK].rearrange(
                    "(o p rt t) -> p o rt t", p=P, rt=TP, t=HTOK)
                agv_v = agkv_in[hf][KBLK:KVBLK].rearrange("(j p v) -> p j v",
                                                          p=P, v=CS)
                for r in range(TP):
                    tsl = slice(r * 512 + hf * HTOK, r * 512 + (hf + 1) * HTOK)
                    q1T = t8.tile([P, KT, HTOK], BF16, tag="t8bf")
                    nc.sync.dma_start_transpose(q1T[:],
                                                rblock_src(ag1_out[hf], r))
                    pk = psp.tile([P, 2, 512], F32, tag="pb2")
                    for o in range(2):
                        for ct in range(KT):
                            nc.tensor.matmul(pk[:, o, 0:HTOK],
                                             wk_bf[:, ct, o * P:(o + 1) * P],
                                             q1T[:, ct, :], start=(ct == 0),
                                             stop=(ct == KT - 1))
                    for o in range(2):
                        dk = t2.tile([P, 512], F32, tag="t2f32")
                        nc.vector.tensor_tensor(dk[:, 0:HTOK], pk[:, o, 0:HTOK],
                                                rinv1_bc[:, tsl], ALU.mult)
                        kq = t1.tile([P, 512], BF16, tag="t1bf", bufs=1)
                        nc.vector.tensor_scalar(kq[:, 0:HTOK], dk[:, 0:HTOK],
                                                bk_col[:, o:o + 1], None,
                                                op0=ALU.add)
                        nc.sync.dma_start(agk_v[:, o, r, :], kq[:, 0:HTOK])
                    pv = psp.tile([P, 2, 512], F32, tag="pb2")
                    for sub in range(2):
                        tt = 4 * r + 2 * hf + sub
                        for ct in range(KT):
                            nc.tensor.matmul(pv[:, sub, 0:CS],
                                             q1T[:, ct, sub * P:(sub + 1) * P],
                                             wv_bf[:, ct, :], start=(ct == 0),
                                             stop=(ct == KT - 1))
                    for sub in range(2):
                        tt = 4 * r + 2 * hf + sub
                        vdq = t1.tile([P, CS], F32, tag="t1f32", bufs=2)
                        nc.vector.tensor_scalar(vdq[:], pv[:, sub, 0:CS],
                                                rinv1_col[:, tt:tt + 1], None,
                                                op0=ALU.mult)
                        vq = t1.tile([P, CS], BF16, tag="t1bfv", bufs=2)
                        nc.vector.tensor_tensor(vq[:], vdq[:], bv_row[:], ALU.add)
                        nc.sync.dma_start(agv_v[:, 2 * r + sub, :], vq[:])
                nc.gpsimd.collective_compute(
                    "AllGather", ALU.bypass, replica_groups=G4,
                    ins=[agkv_in[hf].opt()], outs=[agkv_out[hf].opt()])

            # gather k/v of ALL ranks into kT_all / v_aug (per AG chunk hf)
            v_aug5 = v_aug[:].rearrange("p (rt four) h d -> p rt four h d",
                                        four=4)
            for r in range(TP):      # source rank (channel shard)
                for hf in range(2):
                    base = r * KVBLK
                    ksrc = agkv_out[hf][base:base + KBLK].rearrange(
                        "(o p rt t) -> p o rt t", p=P, rt=TP, t=HTOK)
                    for o in range(2):
                        dst = kT_all[:, 2 * r + o, :].rearrange(
                            "p (rt x) -> p rt x", x=512)[:, :, hf * HTOK:(hf + 1) * HTOK]
                        nc.sync.dma_start(dst, ksrc[:, o, :, :])
                    vsrc = agkv_out[hf][base + KBLK:base + KVBLK].rearrange(
                        "(rt two p h d) -> rt two p h d", two=2, p=P, h=TP, d=DH)
                    for hh in range(TP):
                        for sub in range(2):
                            nc.sync.dma_start(
                                v_aug5[:, :, 2 * hf + sub, TP * r + hh, 0:DH],
                                vsrc[:, sub, :, hh, :].rearrange(
                                    "rt p d -> p rt d"))

            # ---------- attention (16 heads, own 512 query tokens) ----------
            o_un = big.tile([P, TT_LOC, H, DH + 1], BF16, tag="slotO")
            SCALE = DH ** -0.5
            # key tiles in kv-AG-chunk arrival order (chunk 0 tiles first)
            KT_ORDER = [4 * r + 2 * hf + sub for hf in range(2)
                        for r in range(TP) for sub in range(2)]
            # two passes (one per kv AG chunk): each pair's PSUM accumulator
            # is released at the end of its pass, so pass-0 work for all 8
            # pairs streams without waiting for the second kv chunk. Pass 0
            # writes o_un; pass 1 accumulates into it.
            for half_pass in range(2):
                for hp in range(H // 2):
                    h_e, h_o = 2 * hp, 2 * hp + 1
                    po_e = psa.tile([P, 512], F32, tag="po_e")
                    po_o = psa.tile([P, 512], F32, tag="po_o")
                    for kti8 in range(8):
                        tt2 = KT_ORDER[half_pass * 8 + kti8]
                        sreg = psp.tile([P, 2, 512], F32, tag="pb2")
                        for ii, hh in enumerate((h_e, h_o)):
                            jk = DH * hh
                            kT_ap = kT_all[(jk % P):(jk % P) + DH, jk // P,
                                           tt2 * P:(tt2 + 1) * P]
                            qT_ap = qT_own[(jk % P):(jk % P) + DH, jk // P, :]
                            nc.tensor.matmul(sreg[:, ii, :], kT_ap, qT_ap,
                                             start=True, stop=True)
                        pt = t1.tile([P, 2, 512], BF16, tag="ptbf", bufs=3)
                        nc.scalar.activation(pt[:], sreg[:], ACTF.Exp,
                                             scale=SCALE)
                        nc.tensor.matmul(po_e[0:DH + 1, :],
                                         v_aug[:, tt2, h_e, :],
                                         pt[:, 0, :], start=(kti8 == 0),
                                         stop=(kti8 == 7),
                                         skip_group_check=True)
                        nc.tensor.matmul(po_o[0:DH + 1, :],
                                         v_aug[:, tt2, h_o, :],
                                         pt[:, 1, :], start=(kti8 == 0),
                                         stop=(kti8 == 7),
                                         skip_group_check=True)
                    # evacuate this pass's partial into o_un
                    for ii, (po, hh) in enumerate(((po_e, h_e), (po_o, h_o))):
                        stg = t1.tile([DH + 1, 512], BF16, tag="postg", bufs=2)
                        nc.vector.tensor_copy(stg[:], po[0:DH + 1, :])
                        for tb in range(TT_LOC):
                            trp = psp.tile([P, 1024], BF16, tag="pbb")
                            nc.tensor.transpose(trp[:, 0:DH + 1],
                                                stg[:, tb * P:(tb + 1) * P],
                                                ident[0:DH + 1, 0:DH + 1])
                            if half_pass == 0:
                                nc.vector.tensor_copy(o_un[:, tb, hh, :],
                                                      trp[:, 0:DH + 1])
                            else:
                                nc.vector.tensor_tensor(o_un[:, tb, hh, :],
                                                        o_un[:, tb, hh, :],
                                                        trp[:, 0:DH + 1],
                                                        ALU.add)

            # ---------- o quant (fully local) + transpose back ----------
            oqT = qT_own  # reuse slotQ storage (last read: score matmuls)
            mo_col = sm.tile([P, TT_LOC], F32, name="mo_col")
            for tb in range(TT_LOC):
                linv = sm.tile([P, H], BF16, tag="linv")
                with nc.allow_low_precision(reason="1/l feeds int8 quant"):
                    nc.vector.reciprocal(linv[:], o_un[:, tb, :, DH:DH + 1]
                                         .rearrange("p h one -> p (h one)"))
                o_n = t4.tile([P, H, DH], F32, tag="t4f32", name="o_n")
                nc.vector.tensor_tensor(
                    o_n[:], o_un[:, tb, :, 0:DH],
                    linv[:, :, None].to_broadcast((P, H, DH)), ALU.mult)
                nc.vector.tensor_reduce(mo_col[:, tb:tb + 1],
                                        o_n[:].rearrange("p h d -> p (h d)"),
                                        axis=AX.X, op=ALU.max,
                                        apply_absolute_value=True)
                nc.vector.tensor_scalar(mo_col[:, tb:tb + 1],
                                        mo_col[:, tb:tb + 1], EPS, None,
                                        op0=ALU.max)
                so = sm.tile([P, 1], F32, tag="so")
                nc.vector.reciprocal(so[:], mo_col[:, tb:tb + 1])
                nc.vector.tensor_scalar(so[:], so[:], 127.0, None, op0=ALU.mult)
                qtmp = t4.tile([P, C], F32, tag="t4f32", name="qtmp")
                nc.vector.tensor_scalar(qtmp[:],
                                        o_n[:].rearrange("p h d -> p (h d)"),
                                        so[:], MAGIC, op0=ALU.mult, op1=ALU.add)
                oq_tb = t2.tile([P, C], BF16, tag="t2bf")
                nc.vector.tensor_scalar(oq_tb[:], qtmp[:], MAGIC, None,
                                        op0=ALU.subtract)
                for ct in range(KT):
                    trp = psp.tile([P, 1024], BF16, tag="pbb")
                    nc.tensor.transpose(trp[:, 0:P],
                                        oq_tb[:, ct * P:(ct + 1) * P], ident[:])
                    nc.vector.tensor_copy(oqT[:, ct, tb * P:(tb + 1) * P],
                                          trp[:, 0:P])

            # ---------- proj (local, full w_proj) + x_mid ----------
            rinvp_col = sm.tile([P, TT_LOC], F32, name="rinvp_col")
            nc.vector.tensor_scalar(rinvp_col[:], mo_col[:], mean_bc[:, 1:2],
                                    1.0 / 127.0, op0=ALU.mult, op1=ALU.mult)
            bp_row = bcast_row(bp[:], C, "bp_row", pool=brow)
            x_mid = x_res  # accumulate in place
            for tb in range(TT_LOC):
                nc.vector.tensor_tensor(x_mid[:, tb, :], x_mid[:, tb, :],
                                        bp_row[:, :C], ALU.add)
                pp = psp.tile([P, 2, 512], F32, tag="pb2")
                for half in range(2):
                    for ct in range(KT):
                        nc.tensor.matmul(pp[:, half, :],
                                         oqT[:, ct, tb * P:(tb + 1) * P],
                                         wp_bf[:, ct, half * 512:(half + 1) * 512],
                                         start=(ct == 0), stop=(ct == KT - 1))
                for half in range(2):
                    pdq = t2.tile([P, 512], F32, tag="t2f32")
                    nc.vector.tensor_scalar(pdq[:], pp[:, half, :],
                                            rinvp_col[:, tb:tb + 1], None,
                                            op0=ALU.mult)
                    nc.vector.tensor_tensor(x_mid[:, tb, half * 512:(half + 1) * 512],
                                            x_mid[:, tb, half * 512:(half + 1) * 512],
                                            pdq[:], ALU.add)

            # fc weights (loads overlap attention; alias early slots)
            wf1_bf = wres.tile([P, KT, HS], BF16, tag="wslotA")
            wf2_bf = wres.tile([P, KT, C], BF16, tag="wslotC")
            nc.scalar.dma_start(wf1_bf[:],
                                wf1T[:].rearrange("p (o c) -> p o c", c=HS))
            nc.scalar.dma_start(wf2_bf[:],
                                wf2T[:].rearrange("p (o c) -> p o c", c=C))

            # ---------- LN2 + quant + AG2 ----------
            g2_row = be2_row = None
            if not g2_trivial:
                g2_row = bcast_row(g2[:], C, "g2_row", pool=brow)
                be2_row = bcast_row(be2[:], C, "be2_row", pool=brow)
            m2_loc = sm.tile([P, TT_LOC], F32, name="m2_loc")
            for j in range(TT_LOC):
                q2t = t2.tile([P, C], BF16, tag="t2bf")
                ln_quant(x_mid[:, j, :], g2_row, be2_row, g2_trivial, q2t[:],
                         m2_loc[:, j:j + 1])
                nc.sync.dma_start(
                    ag2_in[j // 2][0:HTOK * C]
                    .rearrange("(j p c) -> p j c", p=P, c=C)[:, j % 2, :], q2t[:])
                nc.sync.dma_start(
                    ag2_in[j // 2][HTOK * C:BLK].bitcast(F32)
                    .rearrange("(j p) -> p j", p=P)[:, j % 2:j % 2 + 1],
                    m2_loc[:, j:j + 1])
                if j % 2 == 1:
                    nc.gpsimd.collective_compute(
                        "AllGather", ALU.bypass, replica_groups=G4,
                        ins=[ag2_in[j // 2].opt()],
                        outs=[ag2_out[j // 2].opt()])

            rinv2_col = sm.tile([P, TT_ALL], F32, name="rinv2_col")
            r2c4 = rinv2_col[:].rearrange("p (r four) -> p r four", four=4)
            for hf in range(2):
                for r in range(TP):
                    sc_r = ag2_out[hf][r * BLK + HTOK * C:(r + 1) * BLK].bitcast(F32)
                    joff = r * TT_LOC + hf * 2
                    nc.sync.dma_start(rinv2_col[:, joff:joff + 2],
                                      sc_r.rearrange("(j p) -> p j", p=P))
                nc.vector.tensor_scalar(
                    r2c4[:, :, 2 * hf:2 * hf + 2],
                    r2c4[:, :, 2 * hf:2 * hf + 2],
                    mean_bc[:, 2:3], 1.0 / 127.0, op0=ALU.mult, op1=ALU.mult)

            # ---------- fc1 + gelu (bf16, SBUF-resident) ----------
            bf1_row = bcast_row(bf1[:], HS, "bf1_row", pool=brow)
            bf1_bf = consts.tile([P, HS], BF16, name="bf1_bf")
            nc.vector.tensor_copy(bf1_bf[:], bf1_row[:, :HS])
            gres = kT_all.bitcast(BF16).rearrange("p o n -> p (o n)") \
                .rearrange("p (t h) -> p t h", h=HS)  # alias slotK as [P,16,HS]
            mg_col = sm.tile([P, TT_ALL], F32, name="mg_col")
            # process per AG2 half (hf), per rank block (256 tokens = 2 tiles)
            for hf in range(2):
                for r in range(TP):
                    q2T = t8.tile([P, KT, HTOK], BF16, tag="t8bf")
                    nc.sync.dma_start_transpose(q2T[:],
                                                rblock_src(ag2_out[hf], r))
                    for sub in range(2):
                        tt = r * TT_LOC + hf * 2 + sub
                        gt = gres[:, tt, :]
                        gparts = sm.tile([P, 2], F32, tag="gparts")
                        ph = psp.tile([P, 2, 512], F32, tag="pb2")
                        for half in range(2):
                            for ct in range(KT):
                                nc.tensor.matmul(
                                    ph[:, half, :],
                                    q2T[:, ct, sub * P:(sub + 1) * P],
                                    wf1_bf[:, ct, half * 512:(half + 1) * 512],
                                    start=(ct == 0), stop=(ct == KT - 1))
                        for half in range(2):
                            hsl = slice(half * 512, (half + 1) * 512)
                            gdq = t2.tile([P, 512], BF16, tag="t2bfb")
                            nc.vector.tensor_scalar(gdq[:], ph[:, half, :],
                                                    rinv2_col[:, tt:tt + 1],
                                                    None, op0=ALU.mult)
                            nc.vector.tensor_tensor(gdq[:], gdq[:],
                                                    bf1_bf[:, hsl], ALU.add)
                            nc.scalar.activation(gt[:, hsl], gdq[:], ACTF.Gelu)
                            nc.vector.tensor_reduce(gparts[:, half:half + 1],
                                                    gt[:, hsl], axis=AX.X,
                                                    op=ALU.max,
                                                    apply_absolute_value=True)
                        nc.vector.tensor_reduce(mg_col[:, tt:tt + 1], gparts[:],
                                                axis=AX.X, op=ALU.max)
            nc.vector.tensor_scalar(mg_col[:], mg_col[:], EPS, None, op0=ALU.max)
            nc.sync.dma_start(agg_in[:].rearrange("(j p) -> p j", p=P), mg_col[:])
            nc.gpsimd.collective_compute(
                "AllGather", ALU.bypass, replica_groups=G4,
                ins=[agg_in.opt()], outs=[agg_out.opt()])
            mg_all = sm.tile([P, TT_ALL, TP], F32, name="mg_all")
            for r in range(TP):
                nc.sync.dma_start(
                    mg_all[:, :, r],
                    agg_out[r * N:(r + 1) * N].rearrange("(j p) -> p j", p=P))
            mg_colg = sm.tile([P, TT_ALL], F32, name="mg_colg")
            nc.vector.tensor_reduce(mg_colg[:], mg_all[:], axis=AX.X, op=ALU.max)

            # requant with global scale (in SBUF), spill bf16 ints for fc2
            sg_col = sm.tile([P, TT_ALL], F32, name="sg_col")
            nc.vector.reciprocal(sg_col[:], mg_colg[:])
            nc.vector.tensor_scalar(sg_col[:], sg_col[:], 127.0, None,
                                    op0=ALU.mult)
            for tt in range(TT_ALL):
                qf = t4.tile([P, HS], F32, tag="t4f32")
                nc.vector.tensor_scalar(qf[:], gres[:, tt, :],
                                        sg_col[:, tt:tt + 1], MAGIC,
                                        op0=ALU.mult, op1=ALU.add)
                nc.vector.tensor_scalar(gres[:, tt, :], qf[:], MAGIC, None,
                                        op0=ALU.subtract)
                nc.sync.dma_start(gq_dram[tt * P:(tt + 1) * P, :], gres[:, tt, :])

            # ---------- fc2 (raw int partials, chunked RS) ----------
            for h2 in range(2):
                for r in range(TP):
                    t0 = (TP * r + 2 * h2) * P
                    gT = t8.tile([P, HS // P, HTOK], BF16, tag="t8bf")
                    nc.sync.dma_start_transpose(gT[:], gq_dram[t0:t0 + HTOK, :])
                    for w in range(2):
                        pf = psp.tile([P, 2, 512], F32, tag="pb2")
                        for half in range(2):
                            for ct in range(HS // P):
                                nc.tensor.matmul(
                                    pf[:, half, :], gT[:, ct, w * P:(w + 1) * P],
                                    wf2_bf[:, ct, half * 512:(half + 1) * 512],
                                    start=(ct == 0), stop=(ct == HS // P - 1))
                        fcp = t1.tile([P, 2, 512], BF16, tag="t1bf2", bufs=2)
                        nc.vector.tensor_copy(fcp[:], pf[:])
                        nc.sync.dma_start(
                            rs2h_in[2 * h2 + w][r * P:(r + 1) * P, :],
                            fcp[:].rearrange("p two c -> p (two c)"))
                for w in range(2):
                    nc.gpsimd.collective_compute(
                        "ReduceScatter", ALU.add, replica_groups=G4,
                        ins=[rs2h_in[2 * h2 + w].opt()],
                        outs=[rs2h_out[2 * h2 + w].opt()])

            # ---------- final: y = x_mid + deq(rs2) + bf2 ----------
            bf2_row = bcast_row(bf2[:], C, "bf2_row", pool=brow)
            rinvg_own = sm.tile([P, TT_LOC], F32, name="rinvg_own")
            own_select(rinvg_own[:], mg_colg[:])
            nc.vector.tensor_scalar(rinvg_own[:], rinvg_own[:],
                                    mean_bc[:, 3:4], 1.0 / 127.0,
                                    op0=ALU.mult, op1=ALU.mult)
            for j in range(TT_LOC):
                rst = t2.tile([P, C], BF16, tag="t2bf")
                nc.sync.dma_start(rst[:], rs2h_out[j][:, :])
                yt = t4.tile([P, C], F32, tag="t4f32")
                nc.vector.tensor_scalar(yt[:], rst[:], rinvg_own[:, j:j + 1],
                                        None, op0=ALU.mult)
                nc.vector.tensor_tensor(yt[:], yt[:], bf2_row[:, :C], ALU.add)
                nc.vector.tensor_tensor(yt[:], yt[:], x_mid[:, j, :], ALU.add)
                nc.sync.dma_start(y_sh[j * P:(j + 1) * P, :], yt[:])

    nc.compile()
    return nc


_CACHE = {}


def _ternary(w, scale):
    import ml_dtypes
    return np.clip(np.round(w * scale), -1, 1).astype(ml_dtypes.bfloat16)


def kernel(**inputs):
    m = _imports()
    x = np.ascontiguousarray(np.asarray(inputs["x"]), dtype=np.float32)
    assert int(inputs["num_heads"]) == H
    w_qkv = np.asarray(inputs["w_qkv"], np.float32)
    b_qkv = np.asarray(inputs["b_qkv"], np.float32)
    w_proj = np.asarray(inputs["w_proj"], np.float32)
    b_proj = np.asarray(inputs["b_proj"], np.float32)
    w_fc1 = np.asarray(inputs["w_fc1"], np.float32)
    b_fc1 = np.asarray(inputs["b_fc1"], np.float32)
    w_fc2 = np.asarray(inputs["w_fc2"], np.float32)
    b_fc2 = np.asarray(inputs["b_fc2"], np.float32)
    g1 = np.asarray(inputs["g1"], np.float32)
    be1 = np.asarray(inputs["be1"], np.float32)
    g2 = np.asarray(inputs["g2"], np.float32)
    be2 = np.asarray(inputs["be2"], np.float32)

    g1_trivial = bool(np.all(g1 == 1.0) and np.all(be1 == 0.0))
    g2_trivial = bool(np.all(g2 == 1.0) and np.all(be2 == 0.0))

    key = (g1_trivial, g2_trivial)
    if key not in _CACHE:
        _CACHE[key] = build_kernel(g1_trivial, g2_trivial)
    nc = _CACHE[key]

    # host-side per-tensor absmean scales + ternarization
    means = np.array([max(np.abs(w_qkv).mean(), EPS),
                      max(np.abs(w_proj).mean(), EPS),
                      max(np.abs(w_fc1).mean(), EPS),
                      max(np.abs(w_fc2).mean(), EPS)], np.float32)
    wq_t = _ternary(w_qkv[0:C, :].T, 1.0 / means[0])          # [C, C]
    wk_t = _ternary(w_qkv[C:2 * C, :].T, 1.0 / means[0])      # [C, C]
    wv_t = _ternary(w_qkv[2 * C:, :].T, 1.0 / means[0])       # [C, C]
    wp_t = _ternary(w_proj.T, 1.0 / means[1])                 # [C, C]
    wf1_t = _ternary(w_fc1.T, 1.0 / means[2])                 # [C, HID]
    wf2_t = _ternary(w_fc2.T, 1.0 / means[3])                 # [HID, C]

    def _swz(wT):
        # [(KT*P), cols] -> [P, KT*cols] row-major (device reads it flat)
        cols = wT.shape[1]
        return np.ascontiguousarray(
            wT.reshape(KT, P, cols).transpose(1, 0, 2).reshape(P, KT * cols))

    in_maps = []
    for c in range(NCORES):
        g, r = divmod(c, TP)
        tok = slice(TOK * r, TOK * (r + 1))
        hsl = slice(CS * r, CS * (r + 1))
        im = {
            "x_sh": np.ascontiguousarray(x[g, tok]),
            "wqT": _swz(wq_t),
            "wkT": _swz(wk_t[:, hsl]),
            "wvT": _swz(wv_t[:, hsl]),
            "wpT": _swz(wp_t),
            "wf1T": _swz(wf1_t[:, HS * r:HS * (r + 1)]),
            "wf2T": _swz(wf2_t[HS * r:HS * (r + 1), :]),
            "bq": np.ascontiguousarray(b_qkv[0:C]),
            "bk": np.ascontiguousarray(b_qkv[C:2 * C][hsl]),
            "bv": np.ascontiguousarray(b_qkv[2 * C:][hsl]),
            "bp": b_proj,
            "bf1": np.ascontiguousarray(b_fc1[HS * r:HS * (r + 1)]),
            "bf2": b_fc2,
            "wmean": means,
            "onehot": np.eye(TP, dtype=np.float32)[r],
        }
        if not g1_trivial:
            im["g1"], im["be1"] = g1, be1
        if not g2_trivial:
            im["g2"], im["be2"] = g2, be2
        in_maps.append(im)

    global _last_in_maps
    _last_in_maps = in_maps
    res = m["run"](nc, in_maps, core_ids=list(range(NCORES)))
    out = np.empty((B, N, C), np.float32)
    for c in range(NCORES):
        g, r = divmod(c, TP)
        out[g, TOK * r:TOK * (r + 1)] = res.results[c]["y_sh"]
    return out
